# revision 37
# baseline (speedup 1.0000x reference)
"""Trainium2 Bass kernel for nn_AdditiveAttn (GNN additive-attention message passing).

Strategy: shard edges across 8 cores by *destination node range* (host sorts
edges by dst).  Each core owns N/8 nodes and all edges pointing at them, so
the scatter-softmax and aggregation are core-local (no collectives).  Node
features/weights are replicated.  Host gathers/unpermutes outputs.

Per-core device pipeline (all per 128-edge tile, windows of 128 dst nodes):
  - Ex   = eaT_tile.T @ WE                    (PE, edge-major out)
  - xg   = gather x[src]  (indirect DMA), PE-transpose, @ [WK|WV]
  - Qd   = onehotT.T @ Q_window accumulated into same PSUM as K
  - conn = (K+Qd) + signed_sqrt(E1*E2)   [signed sqrt = exp(.5 ln|s2|) |sign]
  - score = rowsum_16(conn*aw)  clip +-5, ex = exp(score)
  - segment sums via one-hot matmul: psumC += onehot.T @ [V*ex | conn*ex | ex]
  - per-window epilogue: Vo = Q + U/den + (R/den) @ blockdiag(VeRow)
"""

import math
import os
import sys

import numpy as np

sys.path.insert(0, "/opt/trn_rl_repo")

import concourse.bass as bass  # noqa: E402
import concourse.mybir as mybir  # noqa: E402
import concourse.tile as tile  # noqa: E402
from concourse import bacc  # noqa: E402
from concourse.bass import IndirectOffsetOnAxis  # noqa: E402

F32 = mybir.dt.float32
F32R = mybir.dt.float32r
I32 = mybir.dt.int32
AF = mybir.ActivationFunctionType
ALU = mybir.AluOpType

NCORES = 8
P = 128


# ----------------------------------------------------------------------------
# Host-side preprocessing
# ----------------------------------------------------------------------------

def _prep(x, edge_attr, edge_index, WQ, WK, WE, WV, Aw, VeRow):
    N, IN = x.shape
    E = edge_attr.shape[0]
    D, H, _ = Aw.shape
    HD = H * D
    assert IN == P and HD == P, (IN, HD)

    src = edge_index[0].astype(np.int64)
    dst = edge_index[1].astype(np.int64)

    NLOC = (N + NCORES - 1) // NCORES
    NW = (NLOC + P - 1) // P
    NLOC_PAD = NW * P

    order = np.argsort(dst, kind="stable")
    dst_s = dst[order]

    # per-core slices (dst-contiguous)
    bounds = [np.searchsorted(dst_s, c * NLOC) for c in range(NCORES + 1)]

    # First pass: find max tiles-per-window across all cores/windows
    TPW = 1
    core_data = []
    for c in range(NCORES):
        sel = order[bounds[c]:bounds[c + 1]]
        dloc = dst[sel] - c * NLOC
        w = dloc >> 7
        cnt = np.bincount(w, minlength=NW)
        TPW = max(TPW, int(np.ceil(cnt.max() / P)) if len(sel) else 1)
        core_data.append((sel, dloc, w, cnt))
    TPW = ((TPW + 3) // 4) * 4
    NT = NW * TPW
    EPAD = NT * P

    cores = []
    for c in range(NCORES):
        sel, dloc, w, cnt = core_data[c]
        # rank within window
        starts = np.zeros(NW, dtype=np.int64)
        starts[1:] = np.cumsum(cnt)[:-1]
        rank = np.arange(len(sel)) - starts[w]
        slot = w * (TPW * P) + rank
        slot_edge = np.full(EPAD, -1, dtype=np.int64)
        slot_edge[slot] = sel
        valid = slot_edge >= 0
        safe = np.where(valid, slot_edge, 0 if E else 0)

        ea_pad = edge_attr[safe]          # [EPAD, IN]; pad slots get edge 0 (finite)
        eaT = np.ascontiguousarray(ea_pad.T)               # [128, EPAD]
        src_slot = src[safe].astype(np.int32)
        src_m = np.ascontiguousarray(src_slot.reshape(NT, P).T)   # [128, NT]
        offv = np.full(EPAD, -1.0, dtype=np.float32)
        offv[slot] = (dloc - w * P).astype(np.float32)
        off_m = np.ascontiguousarray(offv.reshape(NT, P).T)       # [128, NT] f32

        lo = c * NLOC
        hi = min(lo + NLOC, N)
        xT = np.zeros((P, NLOC_PAD), dtype=np.float32)
        xT[:, : hi - lo] = x[lo:hi].T

        xsT = np.ascontiguousarray(x[src_slot].T)          # [128, EPAD]
        ea32 = eaT.astype(np.float32)
        eaxs = np.concatenate([ea32.reshape(P, NT // 2, 2 * P),
                               xsT.reshape(P, NT // 2, 2 * P)],
                              axis=2).reshape(P, 2 * EPAD)
        cores.append(dict(eaxs=np.ascontiguousarray(eaxs), srcm=src_m,
                          offm=off_m, xTloc=xT, slot_edge=slot_edge,
                          valid=valid, lo=lo, hi=hi))

    # shared constants
    aw_vec = np.empty(P, dtype=np.float32)
    for f in range(P):
        aw_vec[f] = Aw[f % D, f // D, 0]
    awrep = np.tile(aw_vec[None, :], (P, 2))
    iota2 = np.tile(np.arange(P, dtype=np.float32)[None, :], (P, 2)).reshape(P, 2 * P)
    ident = np.eye(P, dtype=np.float32)
    bd = np.zeros((P, P), dtype=np.float32)
    for h in range(H):
        bd[h * D:(h + 1) * D, h * D:(h + 1) * D] = VeRow[:, h, :]
    wkv = np.concatenate([WK, WV], axis=1).astype(np.float32)

    consts = dict(WE=WE.astype(np.float32), WKV=wkv, WQ=WQ.astype(np.float32),
                  AWREP=awrep, IOTA2=iota2, IDENT=ident, IDENTF=ident, BD=bd)
    dims = dict(N=N, E=E, H=H, D=D, NW=NW, TPW=TPW, NT=NT, EPAD=EPAD,
                NLOC=NLOC, NLOC_PAD=NLOC_PAD)
    return cores, consts, dims


# ----------------------------------------------------------------------------
# Device program
# ----------------------------------------------------------------------------

def _patch_act_tables():
    # Reorder activation-table sets so the one covering ALL funcs this kernel
    # uses (copy/abs/sign/ln/exp) is picked for every activation -- otherwise
    # bacc alternates tables and inserts a ~1.3us table load per tile pair.
    import os as _os
    if _os.environ.get("KERNEL_NO_ACT_PATCH"):
        return
    from concourse import bacc as _bacc
    import concourse.hw_specs as _hw
    if getattr(_bacc, "_act_tables_patched", False):
        return
    _orig = _hw.get_activation_tables

    def steered(arch):
        # Keep set order/ids intact (walrus maps act_func_set_id by index),
        # but hide the funcs this kernel uses from every other set so the
        # chooser assigns them all to natural_log_exp_and_others -> one
        # table load for the whole program instead of one per tile pair.
        t = _orig(arch)
        pref = [k for k in t if "natural_log_exp" in k]
        if not pref:
            return t
        nle = t[pref[0]]
        return {k: (v if k in pref else (v - nle)) for k, v in t.items()}

    _bacc.get_activation_tables = steered
    _bacc._act_tables_patched = True


def build_bass(dims, debug=False):
    _patch_act_tables()
    N = dims["N"]
    NW, TPW, NT = dims["NW"], dims["TPW"], dims["NT"]
    EPAD, NLOC_PAD = dims["EPAD"], dims["NLOC_PAD"]
    H, D = dims["H"], dims["D"]

    nc = bacc.Bacc()

    # --- DRAM tensors ------------------------------------------------------
    eaxs_d = nc.dram_tensor("eaxs", [P, 2 * EPAD], F32R, kind="ExternalInput")
    xTloc_d = nc.dram_tensor("xTloc", [P, NLOC_PAD], F32R, kind="ExternalInput")
    offm_d = nc.dram_tensor("offm", [P, NT], F32, kind="ExternalInput")
    WE_d = nc.dram_tensor("WE", [P, 2 * P], F32R, kind="ExternalInput")
    WKV_d = nc.dram_tensor("WKV", [P, 2 * P], F32R, kind="ExternalInput")
    WQ_d = nc.dram_tensor("WQ", [P, P], F32R, kind="ExternalInput")
    AW_d = nc.dram_tensor("AWREP", [P, 2 * P], F32, kind="ExternalInput")
    IO_d = nc.dram_tensor("IOTA2", [P, 2 * P], F32, kind="ExternalInput")
    ID_d = nc.dram_tensor("IDENT", [P, P], F32R, kind="ExternalInput")
    IDF_d = nc.dram_tensor("IDENTF", [P, P], F32, kind="ExternalInput")
    BD_d = nc.dram_tensor("BD", [P, P], F32, kind="ExternalInput")

    conn_d = nc.dram_tensor("conn", [EPAD, P], F32, kind="ExternalOutput")
    vo_d = nc.dram_tensor("vo", [NLOC_PAD, P], F32, kind="ExternalOutput")
    if debug:
        dbg = {k: nc.dram_tensor("dbg_" + k, s, F32, kind="ExternalOutput")
               for k, s in dict(xg=[P, 2 * P], exsb=[P, 2 * P],
                                s2=[P, 2 * P], st=[P, 2 * P], oh=[P, 2 * P],
                                sc8=[P, 2 * 8], comb=[P, 2 * 264],
                                qloc=[P, 2 * P]).items()}

    conn3 = conn_d[:, :].rearrange("(t p) f -> p t f", p=P)

    with tile.TileContext(nc) as tc:
        with (
            tc.tile_pool(name="resident", bufs=1) as rp,
            tc.tile_pool(name="io", bufs=3) as iop,
            tc.tile_pool(name="gath", bufs=2) as gp,
            tc.tile_pool(name="work", bufs=2) as wp,
            tc.tile_pool(name="pA", bufs=2, space="PSUM") as pA,
            tc.tile_pool(name="pB", bufs=2, space="PSUM") as pB,
            tc.tile_pool(name="pT", bufs=2, space="PSUM") as pT,
            tc.tile_pool(name="pC", bufs=1, space="PSUM") as pC,
            tc.tile_pool(name="pM", bufs=1, space="PSUM") as pM,
        ):
            # --- resident loads -------------------------------------------
            def load(dram, shape, tag, dtype=F32):
                t = rp.tile(shape, dtype, tag=tag)
                nc.sync.dma_start(t[:], dram[:, :])
                return t

            WE_s = load(WE_d, [P, 2 * P], "cWE", F32R)
            WKV_s = load(WKV_d, [P, 2 * P], "cWKV", F32R)
            WQ_s = load(WQ_d, [P, P], "cWQ", F32R)
            AW_s = load(AW_d, [P, 2 * P], "cAW")
            IO_s = load(IO_d, [P, 2 * P], "cIO")
            ID_s = load(ID_d, [P, P], "cID", F32R)
            IDF_s = load(IDF_d, [P, P], "cIDF")
            BD_s = load(BD_d, [P, P], "cBD")
            xTl_s = load(xTloc_d, [P, NLOC_PAD], "cxTl", F32R)
            off_s = load(offm_d, [P, NT], "coff")

            eps_s = rp.tile([P, 1], F32, tag="ceps")
            nc.vector.memset(eps_s[:], 1e-38)

            # Q table padded to 256 wide (zeros in upper half) so the Qd
            # matmul can close the same PSUM region the K|V matmul opened.
            qloc = rp.tile([P, NW, 2 * P], F32R, tag="cqloc")

            # --- phase 0: Q table (node-major, windows on free dim) -------
            for w in range(NW):
                pq = pM.tile([P, P], F32, tag="pq")
                nc.tensor.matmul(pq[:], lhsT=xTl_s[:, w * P:(w + 1) * P],
                                 rhs=WQ_s[:], start=True, stop=True)
                nc.scalar.activation(qloc[:, w, 0:P], pq[:], AF.Copy)
                nc.scalar.activation(qloc[:, w, P:2 * P], pq[:], AF.Copy,
                                     scale=0.0)

            # --- phase 1: edge tiles --------------------------------------
            for g in range(NW):
                psC = pC.tile([P, 264], F32, tag="psC")

                for jp in range(TPW // 2):
                    t0 = g * TPW + 2 * jp
                    q = t0 // 2
                    if jp % 2 == 0:
                        eaxs8 = iop.tile([P, 2, 4, P], F32R, tag="eaxs")
                        nc.sync.dma_start(
                            eaxs8[:], eaxs_d[:, q * 4 * P:(q + 2) * 4 * P]
                            .rearrange("p (s j e) -> p s j e", s=2, j=4))
                        cnq = iop.tile([P, 4, P], F32, tag="cnq")
                    sp = jp % 2
                    ea = eaxs8[:, sp, 0:2, :]
                    xgT = eaxs8[:, sp, 2:4, :]

                    oh = wp.tile([P, 2, P], F32R, tag="oh")
                    for j in range(2):
                        nc.gpsimd.tensor_scalar(
                            oh[:, j, :], IO_s[:, 0:P],
                            off_s[:, t0 + j, None], None, ALU.is_equal)

                    psA = pA.tile([P, 2, 2 * P], F32, tag="psA")
                    psB = pB.tile([P, 2, 2 * P], F32, tag="psB")
                    ohT = wp.tile([P, 2, P], F32R, tag="ohT")

                    po = pT.tile([P, 2, P], F32R, tag="pt")
                    for j in range(2):
                        nc.tensor.matmul(psA[:, j, :],
                                         lhsT=ea[:, j, :].bitcast(F32R),
                                         rhs=WE_s[:].bitcast(F32R),
                                         start=True, stop=True)
                        nc.tensor.transpose(po[:, j, :], oh[:, j, :], ID_s[:])
                    nc.scalar.activation(ohT[:], po[:], AF.Copy)
                    for j in range(2):
                        nc.tensor.matmul(psB[:, j, :],
                                         lhsT=xgT[:, j, :].bitcast(F32R),
                                         rhs=WKV_s[:].bitcast(F32R),
                                         start=True, stop=False)
                        nc.tensor.matmul(psB[:, j, :],
                                         lhsT=ohT[:, j, :].bitcast(F32R),
                                         rhs=qloc[:, g, :].bitcast(F32R),
                                         start=False, stop=True)

                    # ---- paired elementwise ------------------------------
                    psA5 = psA[:, :, :].rearrange("p j (h s d) -> p j h s d",
                                                  s=2, d=D)
                    e2sb = wp.tile([P, 2, P], F32, tag="e2sb")
                    nc.scalar.activation(
                        e2sb[:, :, :].rearrange("p j (h d) -> p j h d", d=D),
                        psA5[:, :, :, 1, :], AF.Copy)
                    s2 = wp.tile([P, 2, P], F32, tag="s2")
                    nc.vector.tensor_tensor(
                        s2[:, :, :].rearrange("p j (h d) -> p j h d", d=D),
                        psA5[:, :, :, 0, :],
                        e2sb[:, :, :].rearrange("p j (h d) -> p j h d", d=D),
                        ALU.mult)
                    # signed sqrt: st = s2 * exp(-0.25*ln(s2^2 + eps))
                    #            = sign(s2)*sqrt(|s2|)   (st -> 0 as s2 -> 0)
                    sq = wp.tile([P, 2, P], F32, tag="sq")
                    nc.gpsimd.tensor_tensor(sq[:], s2[:], s2[:], ALU.mult)
                    nc.scalar.activation(sq[:], sq[:], AF.Ln, bias=eps_s[:])
                    nc.scalar.activation(sq[:], sq[:], AF.Exp, scale=-0.25)
                    st = wp.tile([P, 2, P], F32, tag="st")
                    nc.gpsimd.tensor_tensor(st[:], s2[:], sq[:], ALU.mult)
                    # conn = (K + Qd) + st
                    cn = cnq[:, sp * 2:sp * 2 + 2, :]
                    nc.vector.tensor_tensor(cn[:], psB[:, :, 0:P], st[:], ALU.add)
                    if sp == 1:
                        nc.sync.dma_start(conn3[:, t0 - 2:t0 + 2, :], cnq[:])

                    if debug and g == 0 and jp == 0:
                        nc.sync.dma_start(dbg["xg"][:, :], xgT[:].rearrange("p j e -> p (j e)").bitcast(F32))
                        nc.sync.dma_start(dbg["exsb"][:, :], e2sb[:].rearrange("p j e -> p (j e)"))
                        nc.sync.dma_start(dbg["s2"][:, :], s2[:].rearrange("p j e -> p (j e)"))
                        nc.sync.dma_start(dbg["st"][:, :], st[:].rearrange("p j e -> p (j e)"))
                        nc.sync.dma_start(dbg["oh"][:, :], oh[:].rearrange("p j e -> p (j e)").bitcast(F32))
                        nc.sync.dma_start(dbg["qloc"][:, :], qloc[:, 0, :].bitcast(F32))
                    # score
                    scw = wp.tile([P, 2, P], F32, tag="scw")
                    nc.gpsimd.tensor_tensor(
                        scw[:], cn[:],
                        AW_s[:, :].rearrange("p (j e) -> p j e", j=2), ALU.mult)
                    sc8 = wp.tile([P, 2, H], F32, tag="sc8")
                    nc.vector.tensor_reduce(
                        sc8[:],
                        scw[:, :, :].rearrange("p j (h d) -> p j h d", d=D),
                        mybir.AxisListType.X, ALU.add)
                    nc.vector.tensor_scalar(sc8[:], sc8[:], 5.0, -5.0,
                                            ALU.min, ALU.max)
                    comb = wp.tile([P, 2, 264], F32R, tag="comb")
                    nc.scalar.activation(comb[:, :, 256:264], sc8[:], AF.Exp)

                    if debug and g == 0 and jp == 0:
                        nc.sync.dma_start(dbg["sc8"][:, :], sc8[:].rearrange("p j e -> p (j e)"))
                    exb = comb[:, :, 256:264, None].to_broadcast([P, 2, H, D])
                    nc.vector.tensor_tensor(
                        comb[:, :, 0:P].rearrange("p j (h d) -> p j h d", d=D),
                        psB[:, :, P:2 * P].rearrange("p j (h d) -> p j h d", d=D),
                        exb, ALU.mult)
                    nc.vector.tensor_tensor(
                        comb[:, :, P:2 * P].rearrange("p j (h d) -> p j h d", d=D),
                        cn[:, :, :].rearrange("p j (h d) -> p j h d", d=D),
                        exb, ALU.mult)
                    for j in range(2):
                        if debug and g == 0 and jp == 0 and j == 1:
                            nc.sync.dma_start(dbg["comb"][:, :], comb[:].rearrange("p j e -> p (j e)").bitcast(F32))
                        nc.tensor.matmul(psC[:],
                                         lhsT=oh[:, j, :].bitcast(F32R),
                                         rhs=comb[:, j, :].bitcast(F32R),
                                         start=(2 * jp + j == 0),
                                         stop=(2 * jp + j == TPW - 1))

                # ---- phase 2: per-window epilogue ------------------------
                dtmp = wp.tile([P, H], F32, tag="dtmp")
                nc.vector.tensor_scalar(dtmp[:], psC[:, 256:264], 1e-16, None,
                                        ALU.add)
                dinv = wp.tile([P, H], F32, tag="dinv")
                nc.vector.reciprocal(dinv[:], dtmp[:])
                dinvb = dinv[:, :, None].to_broadcast([P, H, D])
                aggv = wp.tile([P, P], F32, tag="aggv")
                nc.vector.tensor_tensor(
                    aggv[:, :].rearrange("p (h d) -> p h d", d=D),
                    psC[:, 0:P].rearrange("p (h d) -> p h d", d=D),
                    dinvb, ALU.mult)
                rv = wp.tile([P, P], F32, tag="rv")
                nc.vector.tensor_tensor(
                    rv[:, :].rearrange("p (h d) -> p h d", d=D),
                    psC[:, P:2 * P].rearrange("p (h d) -> p h d", d=D),
                    dinvb, ALU.mult)
                pt2 = pT.tile([P, P], F32, tag="pt")
                nc.tensor.transpose(pt2[:], rv[:], IDF_s[:])
                rvt = wp.tile([P, P], F32, tag="rvt")
                nc.scalar.activation(rvt[:], pt2[:], AF.Copy)
                psD = pM.tile([P, P], F32, tag="pq")
                nc.tensor.matmul(psD[:], lhsT=rvt[:], rhs=BD_s[:],
                                 start=True, stop=False)
                nc.tensor.matmul(psD[:], lhsT=IDF_s[:], rhs=aggv[:],
                                 start=False, stop=False)
                nc.tensor.matmul(psD[:], lhsT=ID_s[:], rhs=qloc[:, g, 0:P],
                                 start=False, stop=True)
                vo_s = wp.tile([P, P], F32, tag="vo")
                nc.scalar.activation(vo_s[:], psD[:], AF.Copy)
                nc.sync.dma_start(vo_d[g * P:(g + 1) * P, :], vo_s[:])

    nc.compile()
    return nc


# ----------------------------------------------------------------------------
# Entry point
# ----------------------------------------------------------------------------

def kernel(x, edge_attr, edge_index, WQ, bQ, WK, bK, WE, bE, WV, bV, Aw, VeRow):
    x = np.asarray(x, dtype=np.float32)
    edge_attr = np.asarray(edge_attr, dtype=np.float32)
    edge_index = np.asarray(edge_index)
    for b in (bQ, bK, bE, bV):
        assert not np.any(np.asarray(b)), "nonzero biases not supported"

    cores, consts, dims = _prep(x, edge_attr, edge_index,
                                np.asarray(WQ, np.float32), np.asarray(WK, np.float32),
                                np.asarray(WE, np.float32), np.asarray(WV, np.float32),
                                np.asarray(Aw, np.float32), np.asarray(VeRow, np.float32))

    nc = build_bass(dims)

    in_maps = []
    for c in range(NCORES):
        m = dict(eaxs=cores[c]["eaxs"],
                 xTloc=cores[c]["xTloc"], offm=cores[c]["offm"])
        m.update({k: v for k, v in consts.items()})
        in_maps.append(m)

    from concourse import bass_utils
    res = bass_utils.run_bass_kernel_spmd(nc, in_maps, core_ids=list(range(NCORES)))
    global LAST_RESULTS, LAST_NC
    LAST_RESULTS = res
    LAST_NC = nc

    N, E = dims["N"], dims["E"]
    HD = dims["H"] * dims["D"]
    conn_full = np.empty((E, HD), dtype=np.float32)
    vo_full = np.empty((N, HD), dtype=np.float32)
    for c in range(NCORES):
        out = res.results[c]
        se, valid = cores[c]["slot_edge"], cores[c]["valid"]
        conn_full[se[valid]] = out["conn"][valid]
        lo, hi = cores[c]["lo"], cores[c]["hi"]
        vo_full[lo:hi] = out["vo"][: hi - lo]
    return vo_full, conn_full
